# revision 1
# baseline (speedup 1.0000x reference)
"""DiffPathRenderer Trainium2 kernel, v3.

Layout A (partition = pixel row, free = pixel col, one [128,128] image per
segment, running min across segments) with the affine t1 = B*i + (A*j + C)
generated on the TensorEngine: one K=2 matmul per segment,
lhsT = [[i],[1]] (const), rhs = [[B]*128, A*j+C] (host-precomputed).

Per segment:
  t1  = matmul -> PSUM                       PE
  t   = clip(t1, 0, 1)                       DVE (PSUM read)
  m   = -wvx*t + X                           DVE stt
  sx  = (m - vx)^2                           ACT Square (bias col)
  sy  = (-wvy*t + (i-vy))^2                  ACT Square (scale+bias cols)
  w   = sx + sy                              GPSIMD add
  dmin= min(dmin, w)                         alternating DVE / GPSIMD

Finals batched across all 32 strokes (single Sqrt + single Sigmoid table
load), one rearranged DMA out.
"""

import numpy as np

import concourse.bacc as bacc
import concourse.mybir as mybir
import concourse.tile as tile
from concourse.bass_utils import run_bass_kernel_spmd

F32 = mybir.dt.float32
N_CORES = 8
B_TOTAL = 256
B_CORE = B_TOTAL // N_CORES
NSEG = 16
G_CORE = B_CORE * NSEG           # 512
P = 128

# allin columns: 4 coef types x 512 segs | X (128) | invr2, bias7
C_NWVX, C_NVX, C_NWVY, C_IVY = range(4)
X_OFF = 4 * G_CORE
IR_OFF = X_OFF + P
W_ALL = IR_OFF + 2
LH_OFF = IR_OFF + 2              # lhsT const [2,128] on partitions 0-1
W_ALL2 = LH_OFF + P

_cached = {}


def _build_bass():
    nc = bacc.Bacc(None)
    allin = nc.declare_dram_parameter("allin", [P, W_ALL2], F32, isOutput=False)
    rhin = nc.declare_dram_parameter("rhin", [2 * B_CORE, NSEG * P], F32,
                                     isOutput=False)
    out = nc.declare_dram_parameter("out", [B_CORE, P, P], F32, isOutput=True)

    AL = mybir.AluOpType
    AF = mybir.ActivationFunctionType

    with tile.TileContext(nc) as tc:
        with tc.tile_pool(name="main", bufs=1) as cpool, \
             tc.tile_pool(name="work", bufs=4) as wp, \
             tc.tile_pool(name="psum", bufs=4, space="PSUM") as pp:
            atile = cpool.tile([P, W_ALL2], F32, name="atile")
            nc.gpsimd.dma_start(out=atile[:], in_=allin[:])

            def col(cid, g):
                j = cid * G_CORE + g
                return atile[:, j:j + 1]

            xt = atile[:, X_OFF:X_OFF + P]
            irt0 = atile[:, IR_OFF:IR_OFF + 1]
            irt1 = atile[:, IR_OFF + 1:IR_OFF + 2]

            LHS = atile[0:2, LH_OFF:LH_OFF + P]

            dma_ = cpool.tile([P, B_CORE * P], F32, name="dma_")

            for k in range(B_CORE):
                dmin = dma_[:, k * P:(k + 1) * P]
                rh = wp.tile([2, NSEG * P], F32, tag="rh", bufs=3, name="rh")
                nc.sync.dma_start(out=rh[:], in_=rhin[2 * k:2 * k + 2, :])
                for s4 in range(NSEG // 4):
                    # one matmul + one batched clip for 4 segments
                    pt = pp.tile([P, 4 * P], F32, tag="pt", name="pt")
                    nc.tensor.matmul(pt[:], LHS,
                                     rh[:, s4 * 4 * P:(s4 + 1) * 4 * P],
                                     start=True, stop=True)
                    t4 = wp.tile([P, 4 * P], F32, tag="t4", name="t4")
                    nc.vector.tensor_scalar(t4[:], pt[:], 0.0, 1.0, AL.max, AL.min)
                    for ss in range(4):
                        s = s4 * 4 + ss
                        g = k * NSEG + s
                        t = t4[:, ss * P:(ss + 1) * P]
                        m = wp.tile([P, P], F32, tag="m", name="m")
                        nc.vector.scalar_tensor_tensor(m[:], t, col(C_NWVX, g),
                                                       xt, AL.mult, AL.add)
                        sx = wp.tile([P, P], F32, tag="sx", name="sx")
                        nc.scalar.activation(sx[:], m[:], AF.Square,
                                             bias=col(C_NVX, g))
                        sy = wp.tile([P, P], F32, tag="sy", name="sy")
                        nc.scalar.activation(sy[:], t, AF.Square,
                                             bias=col(C_IVY, g),
                                             scale=col(C_NWVY, g))
                        if s == 0:
                            nc.gpsimd.tensor_tensor(dmin, sx[:], sy[:], AL.add)
                        else:
                            w = wp.tile([P, P], F32, tag="w", name="w")
                            nc.gpsimd.tensor_tensor(w[:], sx[:], sy[:], AL.add)
                            nc.vector.tensor_tensor(dmin, dmin, w[:], AL.min)

            # finals
            FB = B_CORE * P
            nc.vector.tensor_scalar_max(dma_[:], dma_[:], 0.0)
            qd = cpool.tile([P, FB], F32, name="qd")
            nc.scalar.activation(qd[:], dma_[:], AF.Sqrt, scale=irt0)
            nc.vector.tensor_scalar_min(qd[:], qd[:], 1.0)
            nc.scalar.activation(dma_[:], qd[:], AF.Sigmoid, scale=-70.0, bias=irt1)
            ov = out[:].rearrange("k i j -> i k j")
            sv = dma_[:].rearrange("i (k j) -> i k j", k=B_CORE)
            nc.sync.dma_start(out=ov, in_=sv)
    nc.finalize()
    return nc


def _host_coefs(traj, thickness):
    traj = np.asarray(traj, dtype=np.float32)
    T = traj * np.float32(128.0)
    v = T[:, :-1]
    w = T[:, 1:]
    wv = w - v
    d = np.sqrt(wv[..., 0] ** 2 + wv[..., 1] ** 2)
    e2 = d * d + np.float32(1e-5)
    inv = np.float32(1.0) / e2
    A = wv[..., 0] * inv                           # (256,16)
    Bc = wv[..., 1] * inv
    C = -(v[..., 0] * wv[..., 0] + v[..., 1] * wv[..., 1]) * inv

    ii = np.arange(P, dtype=np.float32)
    ones = np.ones(P, dtype=np.float32)

    cNWvx = (-wv[..., 0])[..., None] * ones        # (256,16,128)
    cNVx = (-v[..., 0])[..., None] * ones
    cNWvy = (-wv[..., 1])[..., None] * ones
    cIvy = ii - v[..., 1][..., None]

    allc = np.stack([cNWvx, cNVx, cNWvy, cIvy], axis=2)      # (256,16,4,128)
    allc = allc.reshape(N_CORES, G_CORE, 4, P).transpose(0, 3, 2, 1)
    coefs = np.ascontiguousarray(allc.reshape(N_CORES, P, 4 * G_CORE))

    thick = np.float32(np.asarray(thickness))
    r = thick / np.float32(2.0)
    invr2 = np.float32(1.0) / (r * r)

    # rhs rows per segment: [B]*128 ; A*j + C
    jj = np.arange(P, dtype=np.float32)
    r0 = np.broadcast_to(Bc[..., None], Bc.shape + (P,))      # (256,16,128)
    r1 = A[..., None] * jj + C[..., None]

    in_maps = []
    for core in range(N_CORES):
        allin = np.zeros((P, W_ALL2), dtype=np.float32)
        allin[:, 0:4 * G_CORE] = coefs[core]
        allin[:, X_OFF:X_OFF + P] = jj[None, :]
        allin[:, IR_OFF] = invr2
        allin[:, IR_OFF + 1] = np.float32(7.0)
        allin[0:1, LH_OFF:LH_OFF + P] = ii
        allin[1:2, LH_OFF:LH_OFF + P] = 1.0
        rh = np.zeros((2 * B_CORE, NSEG * P), dtype=np.float32)
        k0 = core * B_CORE
        for k in range(B_CORE):
            for s in range(NSEG):
                rh[2 * k, s * P:(s + 1) * P] = r0[k0 + k, s]
                rh[2 * k + 1, s * P:(s + 1) * P] = r1[k0 + k, s]
        in_maps.append({"allin": allin, "rhin": rh})
    return in_maps


def kernel(traj, thickness):
    if "nc" not in _cached:
        _cached["nc"] = _build_bass()
    in_maps = _host_coefs(traj, thickness)
    res = run_bass_kernel_spmd(_cached["nc"], in_maps, list(range(N_CORES)))
    return np.concatenate([res.results[c]["out"] for c in range(N_CORES)], axis=0)



# revision 3
# speedup vs baseline: 1.1171x; 1.1171x over previous
"""DiffPathRenderer Trainium2 kernel, v10 (max-form).

Per segment three affine fields are emitted by bf16 hi/lo K=4 matmuls:
  perp = cross(p - v, wv)/d        (signed perpendicular distance)
  e1   = a - d,  e2 = -a           (a = axial coordinate along wv)
dist ~= max(|perp|, e1, e2)  (L-inf endcap approximation; exact on the
segment slab, sqrt(2) worst case in the endcap wedges which the min over
the polyline mostly masks).

Chain per 4-seg block (FD=512, all batched, no per-segment scalars):
  PE   : 3 matmuls -> PSUM   (PERP bank; E1|E2 two-bank tile)
  ACT  : A  = Abs(PERP)  -> bf16      R1 = Relu(E1) -> bf16
  DVE  : R2 = relu(E2) (ts max0) -> bf16
  DVE  : X = max(A, R1);  W[block] = max(X, R2)     (bf16 2x)
  per stroke: bf16 min tree 2048->128 -> FB
  finals: one ACT Sigmoid(-35*d + 7), rearranged DMA out.
"""

import numpy as np
import ml_dtypes

import concourse.bacc as bacc
import concourse.mybir as mybir
import concourse.tile as tile
from concourse.bass_utils import run_bass_kernel_spmd

F32 = mybir.dt.float32
BF16 = mybir.dt.bfloat16
N_CORES = 8
B_TOTAL = 256
B_CORE = B_TOTAL // N_CORES   # 32 strokes/core
NSEG = 16
NBLK = B_CORE * NSEG // 4     # 128 4-seg blocks/core
P = 128
RHS_W = NBLK * 3 * 512        # rhs columns per core
CHUNK = 8                     # blocks per rhs DMA chunk

_cached = {}


def _build_bass(sig_scale):
    nc = bacc.Bacc(None)
    AL = mybir.AluOpType
    AF = mybir.ActivationFunctionType

    lhs_in = nc.declare_dram_parameter("lhs_in", [4, P], BF16, isOutput=False)
    rhs_in = nc.declare_dram_parameter("rhs_in", [4, RHS_W], BF16, isOutput=False)
    out = nc.declare_dram_parameter("out", [B_CORE, P, P], F32, isOutput=True)

    with tile.TileContext(nc) as tc:
        with tc.tile_pool(name="cst", bufs=1) as cst, \
             tc.tile_pool(name="wk", bufs=2) as wk, \
             tc.tile_pool(name="ps", bufs=2, space="PSUM") as ps:
            LHS = cst.tile([4, P], BF16, name="LHS")
            nc.sync.dma_start(out=LHS[:], in_=lhs_in[:])
            FB = cst.tile([P, B_CORE * P], BF16, name="FB")
            B7 = cst.tile([P, 1], F32, name="B7")
            nc.vector.memset(B7[:], 7.0)

            for k in range(B_CORE):          # stroke
                Wt = wk.tile([P, NSEG * P], BF16, tag="Wt", name="Wt")
                for bb in range(4):          # 4-seg block within stroke
                    b = k * 4 + bb
                    if b % CHUNK == 0:
                        rh = wk.tile([4, CHUNK * 3 * 512], BF16, tag="rh",
                                     bufs=3, name="rh")
                        nc.sync.dma_start(
                            out=rh[:],
                            in_=rhs_in[:, b * 3 * 512:(b + CHUNK) * 3 * 512])
                    o = (b % CHUNK) * 3 * 512
                    psA = ps.tile([P, 512], F32, tag="psA", bufs=3, name="psA")
                    nc.tensor.matmul(psA[:], LHS[:], rh[:, o:o + 512],
                                     start=True, stop=True)
                    psB = ps.tile([P, 1024], F32, tag="psB", name="psB")
                    nc.tensor.matmul(psB[:, 0:512], LHS[:],
                                     rh[:, o + 512:o + 1024],
                                     start=True, stop=True)
                    nc.tensor.matmul(psB[:, 512:1024], LHS[:],
                                     rh[:, o + 1024:o + 1536],
                                     start=True, stop=True)
                    SP = wk.tile([P, 512], BF16, tag="SP", bufs=3, name="SP")
                    nc.scalar.activation(SP[:], psA[:], AF.Square)
                    R1 = wk.tile([P, 512], BF16, tag="R1", bufs=3, name="R1")
                    nc.scalar.activation(R1[:], psB[:, 0:512], AF.Relu)
                    R2 = wk.tile([P, 512], BF16, tag="R2", bufs=3, name="R2")
                    nc.vector.tensor_scalar(R2[:], psB[:, 512:1024], 0.0,
                                            None, AL.max)
                    Ot = wk.tile([P, 512], BF16, tag="Ot", bufs=3, name="Ot")
                    nc.vector.tensor_tensor(Ot[:], R1[:], R2[:], AL.max)
                    O2 = wk.tile([P, 512], BF16, tag="O2", bufs=3, name="O2")
                    nc.scalar.activation(O2[:], Ot[:], AF.Square)
                    nc.vector.tensor_tensor(Wt[:, bb * 512:(bb + 1) * 512],
                                            SP[:], O2[:], AL.add)
                # min tree 2048 -> 128
                m1 = wk.tile([P, 1024], BF16, tag="m1", name="m1")
                nc.vector.tensor_tensor(m1[:], Wt[:, 0:1024], Wt[:, 1024:2048],
                                        AL.min)
                m2 = wk.tile([P, 512], BF16, tag="m2", name="m2")
                nc.vector.tensor_tensor(m2[:], m1[:, 0:512], m1[:, 512:1024],
                                        AL.min)
                m3 = wk.tile([P, 256], BF16, tag="m3", name="m3")
                nc.vector.tensor_tensor(m3[:], m2[:, 0:256], m2[:, 256:512],
                                        AL.min)
                nc.vector.tensor_tensor(FB[:, k * P:(k + 1) * P],
                                        m3[:, 0:128], m3[:, 128:256], AL.min)

            GD = cst.tile([P, B_CORE * P], BF16, name="GD")
            nc.scalar.activation(GD[:], FB[:], AF.Sqrt)
            OUTT = cst.tile([P, B_CORE * P], F32, name="OUTT")
            nc.scalar.activation(OUTT[:], GD[:], AF.Sigmoid,
                                 scale=sig_scale, bias=B7[:])
            ov = out[:].rearrange("k i j -> i k j")
            sv = OUTT[:].rearrange("i (k j) -> i k j", k=B_CORE)
            nc.sync.dma_start(out=ov, in_=sv)
    nc.finalize()
    return nc


def _host_coefs(traj, thickness):
    traj = np.asarray(traj, dtype=np.float64) * 128.0
    v = traj[:, :-1]                     # (256,16,2)
    w = traj[:, 1:]
    wv = w - v
    d = np.sqrt(wv[..., 0] ** 2 + wv[..., 1] ** 2)
    deg = d < 0.5
    ds = np.where(deg, 1.0, d)

    # perp = ( wvy*j - wvx*i + (wvx*vy - wvy*vx) )/d
    cPj = wv[..., 1] / ds
    cPi = -wv[..., 0] / ds
    cP0 = (wv[..., 0] * v[..., 1] - wv[..., 1] * v[..., 0]) / ds
    # a = ( wvx*j + wvy*i - (vx*wvx + vy*wvy) )/d ; e1 = a - d ; e2 = -a
    cAj = wv[..., 0] / ds
    cAi = wv[..., 1] / ds
    cA0 = -(v[..., 0] * wv[..., 0] + v[..., 1] * wv[..., 1]) / ds
    c1j, c1i, c10 = cAj, cAi, cA0 - d
    c2j, c2i, c20 = -cAj, -cAi, -cA0
    # degenerate segment -> point distance decomposition:
    # perp = j - vx ; e1 = i - vy ; e2 = -(i - vy)
    cPj = np.where(deg, 1.0, cPj)
    cPi = np.where(deg, 0.0, cPi)
    cP0 = np.where(deg, -v[..., 0], cP0)
    c1j = np.where(deg, 0.0, c1j)
    c1i = np.where(deg, 1.0, c1i)
    c10 = np.where(deg, -v[..., 1], c10)
    c2j = np.where(deg, 0.0, c2j)
    c2i = np.where(deg, -1.0, c2i)
    c20 = np.where(deg, v[..., 1], c20)

    jj = np.arange(P, dtype=np.float64)
    # X[..., j] = cj*j + c0 per field  -> (256,16,128)
    XP = cPj[..., None] * jj + cP0[..., None]
    X1 = c1j[..., None] * jj + c10[..., None]
    X2 = c2j[..., None] * jj + c20[..., None]

    def hilo(x):
        hi = x.astype(ml_dtypes.bfloat16).astype(np.float64)
        lo = (x - hi).astype(ml_dtypes.bfloat16)
        return hi.astype(ml_dtypes.bfloat16), lo

    # rows [ci_hi, X_hi, ci_lo, X_lo]; ci broadcast over j
    def field_rows(ci, X):
        cihi, cilo = hilo(ci)
        Xhi, Xlo = hilo(X)
        cihi = np.broadcast_to(cihi[..., None], X.shape)
        cilo = np.broadcast_to(cilo[..., None], X.shape)
        return np.stack([cihi, Xhi, cilo, Xlo], axis=0)  # (4,256,16,128)

    RP = field_rows(cPi, XP)
    R1 = field_rows(c1i, X1)
    R2 = field_rows(c2i, X2)
    # assemble rhs: per core, per block (4 segs), fields [P|E1|E2],
    # within field col = s*128 + j
    allf = np.stack([RP, R1, R2], axis=0)          # (3,4,256,16,128)
    allf = allf.reshape(3, 4, N_CORES, B_CORE * NSEG, P)
    rhs = np.empty((N_CORES, 4, NBLK, 3, 4 * P), dtype=ml_dtypes.bfloat16)
    for f in range(3):
        blocks = allf[f].reshape(4, N_CORES, NBLK, 4, P)    # rows,core,blk,s,j
        rhs[:, :, :, f, :] = blocks.transpose(1, 0, 2, 3, 4).reshape(
            N_CORES, 4, NBLK, 4 * P)
    rhs = np.ascontiguousarray(rhs.transpose(0, 1, 2, 3, 4).reshape(
        N_CORES, 4, RHS_W))

    ii = np.arange(P, dtype=np.float64)
    lhs = np.stack([ii, np.ones(P), ii, np.ones(P)]).astype(ml_dtypes.bfloat16)

    in_maps = []
    for core in range(N_CORES):
        in_maps.append({"lhs_in": lhs, "rhs_in": rhs[core]})
    return in_maps


def kernel(traj, thickness):
    # darkness = sigmoid(70*(clip((r-dist)/r,0,1)-0.9)) == sigmoid(7 - (140/t)*dist)
    # up to <1e-12 tail differences outside the clip range.
    sig_scale = -140.0 / float(np.asarray(thickness))
    if _cached.get("scale") != sig_scale:
        _cached["nc"] = _build_bass(sig_scale)
        _cached["scale"] = sig_scale
    in_maps = _host_coefs(traj, thickness)
    res = run_bass_kernel_spmd(_cached["nc"], in_maps, list(range(N_CORES)))
    return np.concatenate([res.results[c]["out"] for c in range(N_CORES)],
                          axis=0)
